# revision 1
# baseline (speedup 1.0000x reference)
"""Sparse avg-pool (segment mean) for Trainium2, 8 NeuronCores — TensorEngine version.

Range-shard coarse ids across cores (core k owns ids [k*31360, (k+1)*31360)),
so no collective is needed.  On each core the segment-sum runs on the
TensorEngine: the host sorts the core's rows by local id and buckets them into
245 windows of 128 consecutive ids, padding each window to `cap` tiles of 128
tokens.  For each 128-token tile the DVE builds a one-hot [token, seg] matrix
(is_equal of the token's window-relative id against an iota row), and the PE
accumulates onehot^T @ [feats | 1] into a per-window [128, 66] PSUM bank in
f32 (bf16 multiplicands: features round once to bf16, counts are exact).  A
DVE epilogue divides sums by max(count, 1) and DMAs the window's 128 output
rows.  No GPSIMD scatter ucode anywhere — the old dma_scatter_add version was
descriptor-generation bound at ~7 ns/token.
"""
import os
import sys
from dataclasses import dataclass

sys.path.insert(0, "/opt/trn_rl_repo")

import numpy as np

NCORES = 8
C = 64
CW = 66  # 64 feats + count + window-relative id
W = 128  # ids per window


@dataclass(frozen=True)
class Cfg:
    n_coarse_pad: int = 250_880  # 8 * 245 * 128
    cap: int = 9                 # tiles of 128 tokens per window
    load_windows: int = 8        # windows per input DMA

    @property
    def rng(self):
        return self.n_coarse_pad // NCORES

    @property
    def n_win(self):  # windows per core
        return self.rng // W

    @property
    def s_slots(self):  # 128-token slots per core
        return self.n_win * self.cap

    @property
    def s_tot(self):
        return self.s_slots * 128


FUSED_OH = bool(int(os.environ.get("KERNEL_FUSED_OH", "1")))

CFG = Cfg()
_nc_cache = {}
LAST_RESULT = None


def build_nc(cfg: Cfg):
    from concourse import bacc, mybir, tile

    bf16 = mybir.dt.bfloat16
    f32 = mybir.dt.float32
    nc = bacc.Bacc("TRN2", target_bir_lowering=False)
    feats_ext = nc.declare_dram_parameter(
        "feats", [128, cfg.s_slots, CW], bf16, isOutput=False
    )
    iota_ext = nc.declare_dram_parameter("iota", [128, W], bf16, isOutput=False)
    out_ext = nc.declare_dram_parameter(
        "out", [cfg.n_win, W, C], f32, isOutput=True
    )

    lw = cfg.load_windows
    n_chunks = (cfg.n_win + lw - 1) // lw
    assert cfg.n_win % lw == 0 or True

    with tile.TileContext(nc) as tc:
        with (
            tc.tile_pool(name="stage", bufs=2) as stagep,
            tc.tile_pool(name="oh", bufs=4) as ohp,
            tc.tile_pool(name="psum", bufs=8, space="PSUM") as psump,
            tc.tile_pool(name="fin", bufs=4) as finp,
            tc.tile_pool(name="cst", bufs=1) as cstp,
        ):
            iota_t = cstp.tile([128, W], bf16)
            nc.sync.dma_start(out=iota_t[:], in_=iota_ext[:])

            for ch in range(n_chunks):
                w0 = ch * lw
                nw = min(lw, cfg.n_win - w0)
                src = stagep.tile([128, lw * cfg.cap, CW], bf16, tag="src")
                nc.sync.dma_start(
                    out=src[:, : nw * cfg.cap, :],
                    in_=feats_ext[:, w0 * cfg.cap : (w0 + nw) * cfg.cap, :],
                )
                for wi in range(nw):
                    w = w0 + wi
                    ps = psump.tile([128, CW], f32, tag="ps")
                    if FUSED_OH:
                        s0 = wi * cfg.cap
                        ohw = ohp.tile([128, cfg.cap, W], bf16, tag="ohw")
                        nc.vector.tensor_tensor(
                            out=ohw[:],
                            in0=src[:, s0 : s0 + cfg.cap, CW - 1 : CW].to_broadcast(
                                [128, cfg.cap, W]
                            ),
                            in1=iota_t[:].unsqueeze(1).to_broadcast(
                                [128, cfg.cap, W]
                            ),
                            op=mybir.AluOpType.is_equal,
                        )
                    for j in range(cfg.cap):
                        s = wi * cfg.cap + j
                        if FUSED_OH:
                            oh = ohw[:, j, :]
                        else:
                            oht = ohp.tile([128, W], bf16, tag="oh")
                            nc.vector.tensor_tensor(
                                out=oht[:],
                                in0=src[:, s, CW - 1 : CW].to_broadcast([128, W]),
                                in1=iota_t[:],
                                op=mybir.AluOpType.is_equal,
                            )
                            oh = oht[:]
                        nc.tensor.matmul(
                            out=ps[:],
                            lhsT=oh,
                            rhs=src[:, s, :CW],
                            start=(j == 0),
                            stop=(j == cfg.cap - 1),
                        )
                    den = finp.tile([128, 1], f32, tag="den")
                    nc.vector.tensor_scalar_max(den[:], ps[:, C : C + 1], 1.0)
                    inv = finp.tile([128, 1], f32, tag="inv")
                    nc.vector.reciprocal(inv[:], den[:])
                    ot = finp.tile([128, C], f32, tag="ot")
                    # multiply on the otherwise-idle ACT engine:
                    # out = Copy(in * scale), scale broadcast per partition
                    nc.scalar.activation(
                        ot[:], ps[:, :C], mybir.ActivationFunctionType.Copy,
                        scale=inv[:],
                    )
                    nc.sync.dma_start(out=out_ext[w], in_=ot[:])
    nc.compile()
    return nc


def shard_inputs(feats, ids, cfg: Cfg):
    """Host: route rows to owner cores, bucket into 128-id windows."""
    import ml_dtypes

    ids = np.asarray(ids, dtype=np.int64).ravel()
    feats = np.asarray(feats, dtype=np.float32)
    owner = ids // cfg.rng
    local = (ids - owner * cfg.rng).astype(np.int32)
    order = np.argsort(owner, kind="stable")
    counts = np.bincount(owner, minlength=NCORES)
    offs = np.zeros(NCORES + 1, np.int64)
    np.cumsum(counts, out=offs[1:])
    feats_sorted = feats[order]
    local_sorted = local[order]

    in_maps = []
    iota = np.broadcast_to(
        np.arange(W, dtype=np.float32), (128, W)
    ).astype(ml_dtypes.bfloat16)
    for k in range(NCORES):
        fk = feats_sorted[offs[k] : offs[k + 1]]
        lk = local_sorted[offs[k] : offs[k + 1]]
        n_k = lk.shape[0]
        fa = np.zeros((cfg.s_tot, CW), np.float32)
        if n_k:
            sorder = np.argsort(lk, kind="stable")
            ls = lk[sorder]
            win = ls >> 7
            wcount = np.bincount(win, minlength=cfg.n_win)
            assert wcount.max() <= cfg.cap * 128, (
                f"window overflow {wcount.max()} > {cfg.cap * 128}"
            )
            wstart = np.zeros(cfg.n_win, np.int64)
            np.cumsum(wcount[:-1], out=wstart[1:])
            rank_in_win = np.arange(n_k) - wstart[win]
            dst = win * (cfg.cap * 128) + rank_in_win
            fa[dst, :C] = fk[sorder]
            fa[dst, C] = 1.0
            fa[dst, C + 1] = (ls & 127).astype(np.float32)
        arranged = np.ascontiguousarray(
            fa.reshape(cfg.s_slots, 128, CW).transpose(1, 0, 2)
        ).astype(ml_dtypes.bfloat16)
        in_maps.append({"feats": arranged, "iota": iota})
    return in_maps


def assemble_output(results, n_coarse, cfg: Cfg):
    out = np.empty((NCORES * cfg.rng, C), np.float32)
    for k in range(NCORES):
        out[k * cfg.rng : (k + 1) * cfg.rng] = results[k]["out"].reshape(
            cfg.rng, C
        )
    return out[:n_coarse]


def emulate_device(in_map, cfg: Cfg):
    feats = np.asarray(in_map["feats"], dtype=np.float32)  # [128, s_slots, CW]
    acc = np.zeros((cfg.n_win, W, CW - 1), np.float64)
    for s in range(cfg.s_slots):
        w = s // cfg.cap
        for p in range(128):
            row = feats[p, s]
            seg = int(row[CW - 1])
            acc[w, seg, :] += row[: CW - 1]
    den = np.maximum(acc[:, :, C], 1.0)[:, :, None]
    return {"out": (acc[:, :, :C] / den).astype(np.float32)}


def _install_axon_hooks_shim():
    """Provide antenv.axon_hooks + the ctypes NTFF hook if the image lacks it.

    Mirrors trn_agent_boot.trn_boot._ntff_profile_via_ctypes so that
    run_bass_kernel_spmd(trace=True) can profile under axon.
    """
    import contextlib
    import ctypes
    import types

    try:
        from antenv.axon_hooks import get_axon_ntff_profile_hook  # noqa: F401

        return
    except ImportError:
        pass
    import antenv

    mod = types.ModuleType("antenv.axon_hooks")
    state = {"h": None}
    mod.set_axon_ntff_profile_hook = lambda h: state.__setitem__("h", h)
    mod.get_axon_ntff_profile_hook = lambda: state["h"]
    antenv.axon_hooks = mod
    sys.modules["antenv.axon_hooks"] = mod

    so_path = "/opt/axon/libaxon_pjrt.so"
    if not os.path.exists(so_path):
        return
    lib = ctypes.CDLL(so_path)
    if not hasattr(lib, "axon_start_nrt_profile"):
        return
    lib.axon_start_nrt_profile.argtypes = [
        ctypes.POINTER(ctypes.c_int64),
        ctypes.c_size_t,
    ]
    lib.axon_start_nrt_profile.restype = ctypes.c_int64
    lib.axon_stop_nrt_profile.argtypes = [ctypes.c_char_p]
    lib.axon_stop_nrt_profile.restype = ctypes.c_int64

    @contextlib.contextmanager
    def _hook(output_dir, device_ids):
        import jax

        jax.devices()
        if device_ids:
            ids = (ctypes.c_int64 * len(device_ids))(*device_ids)
            rc = lib.axon_start_nrt_profile(ids, len(device_ids))
        else:
            rc = lib.axon_start_nrt_profile(None, 0)
        if rc != 0:
            raise RuntimeError(f"axon_start_nrt_profile rc={rc}")
        try:
            yield
        finally:
            n = lib.axon_stop_nrt_profile(str(output_dir).encode())
            print(f"profile: {n} file(s) written to {output_dir}", file=sys.stderr)

    state["h"] = _hook


def kernel(fine_feats, coarse_ids, num_coarse):
    global LAST_RESULT
    from concourse.bass_utils import run_bass_kernel_spmd

    cfg = CFG
    # adapt window capacity to the data (stays at the default for the
    # expected uniform-random ids; protects other distributions)
    ids64 = np.asarray(coarse_ids, dtype=np.int64).ravel()
    owner = ids64 // cfg.rng
    local = ids64 - owner * cfg.rng
    mx = 0
    for k in range(NCORES):
        lk = local[owner == k]
        if lk.size:
            mx = max(mx, int(np.bincount(lk >> 7, minlength=cfg.n_win).max()))
    need_cap = max(cfg.cap, -(-mx // 128))
    if need_cap != cfg.cap:
        cfg = Cfg(cap=need_cap)
    in_maps = shard_inputs(fine_feats, coarse_ids, cfg)
    key = ("full", cfg.cap)
    if key not in _nc_cache:
        _nc_cache[key] = build_nc(cfg)
    nc = _nc_cache[key]
    trace = bool(int(os.environ.get("KERNEL_TRACE", "0")))
    if trace:
        _install_axon_hooks_shim()
    res = run_bass_kernel_spmd(nc, in_maps, core_ids=list(range(NCORES)), trace=trace)
    LAST_RESULT = res
    return assemble_output(res.results, int(num_coarse), cfg)



# revision 6
# speedup vs baseline: 1.3119x; 1.3119x over previous
"""Sparse avg-pool (segment mean) for Trainium2, 8 NeuronCores — v2.

Range-shard coarse ids across cores (core k owns 31360 consecutive ids), so no
collective is needed.  On each core the segment-sum runs on the TensorEngine
via one-hot matmuls, with three structural changes vs the old version:

1. Pair fold in the DMA: the host pairs same-segment tokens; the even members
   are DMA'd normally (HWDGE) and the odd members are DMA'd with
   accum_op=add (SWDGE CCE inline add) onto the same SBUF buffer.  The
   pair-sum costs zero engine time and halves the slot count (2205 -> 1225).
2. One-hot built with tensor_scalar(is_equal) against a materialized iota row,
   with the pair's window-relative id as the per-partition f32 scalar.  This
   runs in the DVE's fast mode, unlike the old broadcast tensor_tensor which
   was stuck at 1x (334 us of DVE time, the kernel's bottleneck).
3. Counts are computed on the host (bincount); the device epilogue is a single
   ACT activation Copy with per-partition scale = 1/max(count,1), writing bf16
   (the old DVE max + reciprocal epilogue is gone, and the output DMA halves).
"""
import os
import sys
from dataclasses import dataclass

sys.path.insert(0, "/opt/trn_rl_repo")

import numpy as np

NCORES = 8
C = 64
W = 128  # segment ids per window


@dataclass(frozen=True)
class Cfg:
    n_coarse_pad: int = 250_880  # 8 * 245 * 128
    capp: int = 5                # pair slots (of 128 pairs) per window
    chunk_windows: int = 35      # windows per staged input chunk

    @property
    def rng(self):
        return self.n_coarse_pad // NCORES

    @property
    def n_win(self):  # windows per core
        return self.rng // W

    @property
    def slots(self):  # pair slots per core
        return self.n_win * self.capp


CFG = Cfg()
_nc_cache = {}
LAST_RESULT = None

# how the odd pair members get folded onto the evens:
#   "dma": SWDGE accum_op=add during the odds DMA (CCE inline add)
#   "dve": separate odds buffer + DVE tensor_tensor add
PAIR_MODE = os.environ.get("KERNEL_PAIR_MODE", "dve")


def build_nc(cfg: Cfg):
    from concourse import bacc, mybir, tile

    bf16 = mybir.dt.bfloat16
    f32 = mybir.dt.float32
    nc = bacc.Bacc("TRN2", target_bir_lowering=False)
    evens_ext = nc.declare_dram_parameter("evens", [128, cfg.slots, C], bf16, isOutput=False)
    odds_ext = nc.declare_dram_parameter("odds", [128, cfg.slots, C], bf16, isOutput=False)
    idsrel_ext = nc.declare_dram_parameter("idsrel", [128, cfg.slots], f32, isOutput=False)
    inv_ext = nc.declare_dram_parameter("inv", [128, cfg.n_win], f32, isOutput=False)
    iota_ext = nc.declare_dram_parameter("iota", [128, W], bf16, isOutput=False)
    out_ext = nc.declare_dram_parameter("out", [128, cfg.n_win, C], bf16, isOutput=True)

    ch_w = cfg.chunk_windows
    assert cfg.n_win % ch_w == 0
    n_chunks = cfg.n_win // ch_w
    ch_slots = ch_w * cfg.capp

    with tile.TileContext(nc) as tc:
        with (
            tc.tile_pool(name="cst", bufs=1) as cstp,
            tc.tile_pool(name="stage", bufs=2) as stagep,
            tc.tile_pool(name="oh", bufs=4) as ohp,
            tc.tile_pool(name="psum", bufs=8, space="PSUM") as psump,
            tc.tile_pool(name="ost", bufs=2) as outp,
        ):
            iota_t = cstp.tile([128, W], bf16)
            nc.sync.dma_start(out=iota_t[:], in_=iota_ext[:])
            idsrel_t = cstp.tile([128, cfg.slots], f32)
            nc.sync.dma_start(out=idsrel_t[:], in_=idsrel_ext[:])
            inv_t = cstp.tile([128, cfg.n_win], f32)
            nc.sync.dma_start(out=inv_t[:], in_=inv_ext[:])

            for ch in range(n_chunks):
                s0 = ch * ch_slots
                buf = stagep.tile([128, ch_slots, C], bf16, tag="buf")
                nc.sync.dma_start(
                    out=buf[:], in_=evens_ext[:, s0 : s0 + ch_slots, :]
                )
                if PAIR_MODE == "dma":
                    nc.gpsimd.dma_start(
                        out=buf[:],
                        in_=odds_ext[:, s0 : s0 + ch_slots, :],
                        accum_op=mybir.AluOpType.add,
                    )
                else:
                    obuf = stagep.tile([128, ch_slots, C], bf16, tag="obuf")
                    nc.sync.dma_start(
                        out=obuf[:], in_=odds_ext[:, s0 : s0 + ch_slots, :]
                    )
                    nc.vector.tensor_tensor(
                        out=buf[:], in0=buf[:], in1=obuf[:],
                        op=mybir.AluOpType.add,
                    )
                ostage = outp.tile([128, ch_w, C], bf16, tag="ostage")
                for wl in range(ch_w):
                    w = ch * ch_w + wl
                    oh = ohp.tile([128, cfg.capp, W], bf16, tag="oh")
                    ps = psump.tile([128, C], f32, tag="ps")
                    for j in range(cfg.capp):
                        nc.vector.tensor_scalar(
                            out=oh[:, j, :],
                            in0=iota_t[:],
                            scalar1=idsrel_t[:, w * cfg.capp + j : w * cfg.capp + j + 1],
                            scalar2=None,
                            op0=mybir.AluOpType.is_equal,
                        )
                        nc.tensor.matmul(
                            out=ps[:],
                            lhsT=oh[:, j, :],
                            rhs=buf[:, wl * cfg.capp + j, :],
                            start=(j == 0),
                            stop=(j == cfg.capp - 1),
                        )
                    nc.scalar.activation(
                        ostage[:, wl, :], ps[:],
                        mybir.ActivationFunctionType.Copy,
                        scale=inv_t[:, w : w + 1],
                    )
                nc.sync.dma_start(
                    out=out_ext[:, ch * ch_w : (ch + 1) * ch_w, :], in_=ostage[:]
                )
    nc.compile()
    return nc


def shard_inputs(feats, ids, cfg: Cfg):
    """Host: route rows to owner cores, pair same-segment tokens, bucket pairs
    into static capp-slot windows of 128 consecutive ids."""
    import ml_dtypes

    ids = np.asarray(ids, dtype=np.int64).ravel()
    feats = np.asarray(feats, dtype=np.float32)
    owner = ids // cfg.rng
    local = (ids - owner * cfg.rng).astype(np.int64)
    order = np.argsort(owner, kind="stable")
    counts_core = np.bincount(owner, minlength=NCORES)
    offs = np.zeros(NCORES + 1, np.int64)
    np.cumsum(counts_core, out=offs[1:])
    feats_sorted = feats[order]
    local_sorted = local[order]

    iota = np.broadcast_to(
        np.arange(W, dtype=np.float32), (128, W)
    ).astype(ml_dtypes.bfloat16)

    in_maps = []
    need_capp = cfg.capp
    for k in range(NCORES):
        fk = feats_sorted[offs[k] : offs[k + 1]]
        lk = local_sorted[offs[k] : offs[k + 1]]
        n_k = lk.shape[0]
        evens = np.zeros((128, cfg.slots, C), np.float32)
        odds = np.zeros((128, cfg.slots, C), np.float32)
        idsrel = np.full((128, cfg.slots), -1.0, np.float32)
        cnt = np.bincount(lk, minlength=cfg.rng) if n_k else np.zeros(cfg.rng, np.int64)
        if n_k:
            sorder = np.argsort(lk, kind="stable")
            ls = lk[sorder]
            fs = fk[sorder]
            p_s = (cnt + 1) // 2                       # pairs per seg [rng]
            P_w = p_s.reshape(cfg.n_win, W).sum(1)     # pairs per window
            mx = int(P_w.max())
            if mx > cfg.capp * 128:
                need_capp = max(need_capp, -(-mx // 128))
                in_maps.append(None)
                continue
            pp = p_s.reshape(cfg.n_win, W)
            pair_base = (np.cumsum(pp, axis=1) - pp).ravel()  # excl cumsum in window
            seg_start = np.cumsum(cnt) - cnt
            rho = np.arange(n_k) - seg_start[ls]       # rank within seg
            member = (rho & 1).astype(np.int64)        # 0 = even, 1 = odd
            pair_in_win = pair_base[ls] + (rho >> 1)
            slot_glob = (ls >> 7) * cfg.capp + (pair_in_win >> 7)
            part = pair_in_win & 127
            ev = member == 0
            od = ~ev
            evens[part[ev], slot_glob[ev], :] = fs[ev]
            odds[part[od], slot_glob[od], :] = fs[od]
            idsrel[part[ev], slot_glob[ev]] = (ls[ev] & 127).astype(np.float32)
        inv = 1.0 / np.maximum(cnt, 1).astype(np.float32)
        in_maps.append({
            "evens": evens.astype(ml_dtypes.bfloat16),
            "odds": odds.astype(ml_dtypes.bfloat16),
            "idsrel": idsrel,
            "inv": np.ascontiguousarray(
                inv.reshape(cfg.n_win, W).T
            ),  # [128 seg-in-window, n_win]
            "iota": iota,
        })
    return in_maps, need_capp


def assemble_output(results, n_coarse, cfg: Cfg):
    out = np.empty((NCORES * cfg.rng, C), np.float32)
    for k in range(NCORES):
        o = np.asarray(results[k]["out"], dtype=np.float32)  # [128, n_win, C]
        out[k * cfg.rng : (k + 1) * cfg.rng] = o.transpose(1, 0, 2).reshape(
            cfg.rng, C
        )
    return out[:n_coarse]


def emulate_device(in_map, cfg: Cfg):
    """Pure-numpy model of the device program, for testing the packing."""
    evens = np.asarray(in_map["evens"], dtype=np.float32)
    odds = np.asarray(in_map["odds"], dtype=np.float32)
    idsrel = in_map["idsrel"]
    inv = in_map["inv"]  # [128, n_win]
    import ml_dtypes
    summ = (evens + odds).astype(ml_dtypes.bfloat16).astype(np.float32)
    out = np.zeros((128, cfg.n_win, C), np.float32)
    for w in range(cfg.n_win):
        acc = np.zeros((W, C), np.float64)
        for j in range(cfg.capp):
            s = w * cfg.capp + j
            for p in range(128):
                r = idsrel[p, s]
                if 0 <= r < W:
                    acc[int(r)] += summ[p, s]
        out[:, w, :] = acc * inv[:, w][:, None]
    return {"out": out.astype(ml_dtypes.bfloat16)}


def _install_axon_hooks_shim():
    """Provide antenv.axon_hooks + the ctypes NTFF hook if the image lacks it."""
    import contextlib
    import ctypes
    import types

    try:
        from antenv.axon_hooks import get_axon_ntff_profile_hook  # noqa: F401

        return
    except ImportError:
        pass
    import antenv

    mod = types.ModuleType("antenv.axon_hooks")
    state = {"h": None}
    mod.set_axon_ntff_profile_hook = lambda h: state.__setitem__("h", h)
    mod.get_axon_ntff_profile_hook = lambda: state["h"]
    antenv.axon_hooks = mod
    sys.modules["antenv.axon_hooks"] = mod

    so_path = "/opt/axon/libaxon_pjrt.so"
    if not os.path.exists(so_path):
        return
    lib = ctypes.CDLL(so_path)
    if not hasattr(lib, "axon_start_nrt_profile"):
        return
    lib.axon_start_nrt_profile.argtypes = [
        ctypes.POINTER(ctypes.c_int64),
        ctypes.c_size_t,
    ]
    lib.axon_start_nrt_profile.restype = ctypes.c_int64
    lib.axon_stop_nrt_profile.argtypes = [ctypes.c_char_p]
    lib.axon_stop_nrt_profile.restype = ctypes.c_int64

    @contextlib.contextmanager
    def _hook(output_dir, device_ids):
        import jax

        jax.devices()
        if device_ids:
            ids = (ctypes.c_int64 * len(device_ids))(*device_ids)
            rc = lib.axon_start_nrt_profile(ids, len(device_ids))
        else:
            rc = lib.axon_start_nrt_profile(None, 0)
        if rc != 0:
            raise RuntimeError(f"axon_start_nrt_profile rc={rc}")
        try:
            yield
        finally:
            n = lib.axon_stop_nrt_profile(str(output_dir).encode())
            print(f"profile: {n} file(s) written to {output_dir}", file=sys.stderr)

    state["h"] = _hook


def kernel(fine_feats, coarse_ids, num_coarse):
    global LAST_RESULT
    from concourse.bass_utils import run_bass_kernel_spmd

    num_coarse = int(num_coarse)
    pad = NCORES * W
    n_pad = max(250_880, -(-num_coarse // pad) * pad)
    # keep windows divisible by chunk_windows
    cfg = Cfg(n_coarse_pad=n_pad)
    while cfg.n_win % cfg.chunk_windows != 0:
        n_pad += pad
        cfg = Cfg(n_coarse_pad=n_pad)

    in_maps, need_capp = shard_inputs(fine_feats, coarse_ids, cfg)
    while need_capp != cfg.capp:
        cfg = Cfg(n_coarse_pad=cfg.n_coarse_pad, capp=need_capp)
        in_maps, need_capp = shard_inputs(fine_feats, coarse_ids, cfg)

    key = ("v2", PAIR_MODE, cfg.n_coarse_pad, cfg.capp)
    if key not in _nc_cache:
        _nc_cache[key] = build_nc(cfg)
    nc = _nc_cache[key]
    trace = bool(int(os.environ.get("KERNEL_TRACE", "0")))
    if trace:
        _install_axon_hooks_shim()
    res = run_bass_kernel_spmd(nc, in_maps, core_ids=list(range(NCORES)), trace=trace)
    LAST_RESULT = res
    return assemble_output(res.results, num_coarse, cfg)


# revision 7
# speedup vs baseline: 2.1745x; 1.6575x over previous
"""Sparse avg-pool (segment mean) for Trainium2, 8 NeuronCores — v3.

Range-shard coarse ids across cores (core k owns 31360 consecutive ids), so no
collective is needed.  Each core segment-sums its shard on the TensorEngine.

Key structure ("identity placement"): the host pairs same-segment tokens and
places pair j (j<4) of segment r at PARTITION r of pair-slot j — so those four
slots per window accumulate into PSUM with a constant identity stationary and
need no per-slot one-hot at all.  Only overflow pairs (segments with more than
8 tokens, ~86 per 128-id window) land in a 5th slot at arbitrary partitions
with a real one-hot, built once per window (245 broadcast is_equal builds
instead of 1225 — the DVE was the bottleneck of every earlier version).

The odd pair members are DMA'd into a separate buffer and folded onto the
evens with one big DVE tensor_tensor add per chunk (bf16 dense = 2x mode).
Counts are computed on the host; the epilogue is one ACT Copy per window with
per-partition scale = 1/max(count,1), writing bf16 (halves the output DMA).
"""
import os
import sys
from dataclasses import dataclass

sys.path.insert(0, "/opt/trn_rl_repo")

import numpy as np

NCORES = 8
C = 64
W = 128      # segment ids per window
B = 4        # identity-placed pair slots per window


@dataclass(frozen=True)
class Cfg:
    n_coarse_pad: int = 250_880  # 8 * 245 * 128
    n_ov: int = 1                # overflow pair slots per window
    chunk_windows: int = 35      # windows per staged input chunk

    @property
    def capp(self):
        return B + self.n_ov

    @property
    def rng(self):
        return self.n_coarse_pad // NCORES

    @property
    def n_win(self):  # windows per core
        return self.rng // W

    @property
    def slots(self):  # pair slots per core
        return self.n_win * self.capp


CFG = Cfg()
_nc_cache = {}
LAST_RESULT = None


def build_nc(cfg: Cfg):
    from concourse import bacc, mybir, tile

    bf16 = mybir.dt.bfloat16
    f32 = mybir.dt.float32
    nc = bacc.Bacc("TRN2", target_bir_lowering=False)
    evens_ext = nc.declare_dram_parameter("evens", [128, cfg.slots, C], bf16, isOutput=False)
    odds_ext = nc.declare_dram_parameter("odds", [128, cfg.slots, C], bf16, isOutput=False)
    idsrel_ext = nc.declare_dram_parameter(
        "idsrel", [128, cfg.n_win, cfg.n_ov], f32, isOutput=False
    )
    inv_ext = nc.declare_dram_parameter("inv", [128, cfg.n_win], f32, isOutput=False)
    iota_ext = nc.declare_dram_parameter("iota", [128, W], bf16, isOutput=False)
    ident_ext = nc.declare_dram_parameter("ident", [128, W], bf16, isOutput=False)
    out_ext = nc.declare_dram_parameter("out", [128, cfg.n_win, C], bf16, isOutput=True)

    ch_w = cfg.chunk_windows
    assert cfg.n_win % ch_w == 0
    n_chunks = cfg.n_win // ch_w
    ch_slots = ch_w * cfg.capp

    with tile.TileContext(nc) as tc:
        with (
            tc.tile_pool(name="cst", bufs=1) as cstp,
            tc.tile_pool(name="stage", bufs=2) as stagep,
            tc.tile_pool(name="oh", bufs=2) as ohp,
            tc.tile_pool(name="psum", bufs=8, space="PSUM") as psump,
            tc.tile_pool(name="ost", bufs=2) as outp,
        ):
            iota_t = cstp.tile([128, W], bf16)
            nc.sync.dma_start(out=iota_t[:], in_=iota_ext[:])
            ident_t = cstp.tile([128, W], bf16)
            nc.sync.dma_start(out=ident_t[:], in_=ident_ext[:])
            idsrel_t = cstp.tile([128, cfg.n_win, cfg.n_ov], f32)
            nc.sync.dma_start(out=idsrel_t[:], in_=idsrel_ext[:])
            inv_t = cstp.tile([128, cfg.n_win], f32)
            nc.sync.dma_start(out=inv_t[:], in_=inv_ext[:])

            for ch in range(n_chunks):
                s0 = ch * ch_slots
                buf = stagep.tile([128, ch_slots, C], bf16, tag="buf")
                nc.sync.dma_start(
                    out=buf[:], in_=evens_ext[:, s0 : s0 + ch_slots, :]
                )
                obuf = stagep.tile([128, ch_slots, C], bf16, tag="obuf")
                nc.sync.dma_start(
                    out=obuf[:], in_=odds_ext[:, s0 : s0 + ch_slots, :]
                )
                nc.vector.tensor_tensor(
                    out=buf[:], in0=buf[:], in1=obuf[:], op=mybir.AluOpType.add
                )
                # one-hots for this chunk's overflow slots, one batched build
                oh = ohp.tile([128, ch_w, cfg.n_ov, W], bf16, tag="oh")
                nc.vector.tensor_tensor(
                    out=oh[:],
                    in0=idsrel_t[:, ch * ch_w : (ch + 1) * ch_w, :]
                    .unsqueeze(3)
                    .to_broadcast([128, ch_w, cfg.n_ov, W]),
                    in1=iota_t[:]
                    .unsqueeze(1)
                    .unsqueeze(1)
                    .to_broadcast([128, ch_w, cfg.n_ov, W]),
                    op=mybir.AluOpType.is_equal,
                )
                ostage = outp.tile([128, ch_w, C], bf16, tag="ostage")
                for wl in range(ch_w):
                    w = ch * ch_w + wl
                    ps = psump.tile([128, C], f32, tag="ps")
                    for j in range(B):
                        nc.tensor.matmul(
                            out=ps[:],
                            lhsT=ident_t[:],
                            rhs=buf[:, wl * cfg.capp + j, :],
                            start=(j == 0),
                            stop=False,
                        )
                    for v in range(cfg.n_ov):
                        nc.tensor.matmul(
                            out=ps[:],
                            lhsT=oh[:, wl, v, :],
                            rhs=buf[:, wl * cfg.capp + B + v, :],
                            start=False,
                            stop=(v == cfg.n_ov - 1),
                        )
                    nc.scalar.activation(
                        ostage[:, wl, :], ps[:],
                        mybir.ActivationFunctionType.Copy,
                        scale=inv_t[:, w : w + 1],
                    )
                nc.sync.dma_start(
                    out=out_ext[:, ch * ch_w : (ch + 1) * ch_w, :], in_=ostage[:]
                )
    nc.compile()
    return nc


def shard_inputs(feats, ids, cfg: Cfg):
    """Host: route rows to owner cores, pair same-segment tokens.  Pairs 0..3
    of segment r go to partition r of identity slots 0..3; overflow pairs fill
    the ov slots densely with a window-relative id for the one-hot."""
    import ml_dtypes

    ids = np.asarray(ids, dtype=np.int64).ravel()
    feats = np.asarray(feats, dtype=np.float32)
    owner = ids // cfg.rng
    local = (ids - owner * cfg.rng).astype(np.int64)
    order = np.argsort(owner, kind="stable")
    counts_core = np.bincount(owner, minlength=NCORES)
    offs = np.zeros(NCORES + 1, np.int64)
    np.cumsum(counts_core, out=offs[1:])
    feats_sorted = feats[order]
    local_sorted = local[order]

    iota = np.broadcast_to(
        np.arange(W, dtype=np.float32), (128, W)
    ).astype(ml_dtypes.bfloat16)
    ident = np.eye(W, dtype=np.float32).astype(ml_dtypes.bfloat16)

    in_maps = []
    need_ov = cfg.n_ov
    for k in range(NCORES):
        fk = feats_sorted[offs[k] : offs[k + 1]]
        lk = local_sorted[offs[k] : offs[k + 1]]
        n_k = lk.shape[0]
        evens = np.zeros((128, cfg.slots, C), np.float32)
        odds = np.zeros((128, cfg.slots, C), np.float32)
        idsrel = np.full((128, cfg.n_win, cfg.n_ov), -1.0, np.float32)
        cnt = np.bincount(lk, minlength=cfg.rng) if n_k else np.zeros(cfg.rng, np.int64)
        if n_k:
            sorder = np.argsort(lk, kind="stable")
            ls = lk[sorder]
            fs = fk[sorder]
            p_s = (cnt + 1) // 2                     # pairs per seg
            ov_s = np.maximum(p_s - B, 0)            # overflow pairs per seg
            OV_w = ov_s.reshape(cfg.n_win, W).sum(1)
            mx = int(OV_w.max())
            if mx > cfg.n_ov * 128:
                need_ov = max(need_ov, -(-mx // 128))
                in_maps.append(None)
                continue
            ovp = ov_s.reshape(cfg.n_win, W)
            ov_base = (np.cumsum(ovp, axis=1) - ovp).ravel()  # excl cumsum in window
            seg_start = np.cumsum(cnt) - cnt
            rho = np.arange(n_k) - seg_start[ls]     # rank within seg
            member = (rho & 1).astype(np.int64)      # 0 = even, 1 = odd
            pidx = rho >> 1                          # pair index within seg
            win = ls >> 7
            r = ls & 127
            is_id = pidx < B
            # identity region: partition r, slot = win*capp + pidx
            slot_glob = np.where(
                is_id,
                win * cfg.capp + pidx,
                win * cfg.capp + B + ((ov_base[ls] + pidx - B) >> 7),
            )
            part = np.where(is_id, r, (ov_base[ls] + pidx - B) & 127)
            ev = member == 0
            od = ~ev
            evens[part[ev], slot_glob[ev], :] = fs[ev]
            odds[part[od], slot_glob[od], :] = fs[od]
            ovv = ev & ~is_id
            idsrel[
                part[ovv],
                win[ovv],
                (ov_base[ls[ovv]] + pidx[ovv] - B) >> 7,
            ] = r[ovv].astype(np.float32)
        inv = 1.0 / np.maximum(cnt, 1).astype(np.float32)
        in_maps.append({
            "evens": evens.astype(ml_dtypes.bfloat16),
            "odds": odds.astype(ml_dtypes.bfloat16),
            "idsrel": idsrel,
            "inv": np.ascontiguousarray(
                inv.reshape(cfg.n_win, W).T
            ),  # [128 seg-in-window, n_win]
            "iota": iota,
            "ident": ident,
        })
    return in_maps, need_ov


def assemble_output(results, n_coarse, cfg: Cfg):
    out = np.empty((NCORES * cfg.rng, C), np.float32)
    for k in range(NCORES):
        o = np.asarray(results[k]["out"], dtype=np.float32)  # [128, n_win, C]
        out[k * cfg.rng : (k + 1) * cfg.rng] = o.transpose(1, 0, 2).reshape(
            cfg.rng, C
        )
    return out[:n_coarse]


def emulate_device(in_map, cfg: Cfg):
    """Pure-numpy model of the device program, for testing the packing."""
    import ml_dtypes
    evens = np.asarray(in_map["evens"], dtype=np.float32)
    odds = np.asarray(in_map["odds"], dtype=np.float32)
    idsrel = in_map["idsrel"]
    inv = in_map["inv"]  # [128, n_win]
    summ = (evens + odds).astype(ml_dtypes.bfloat16).astype(np.float32)
    out = np.zeros((128, cfg.n_win, C), np.float32)
    for w in range(cfg.n_win):
        acc = np.zeros((W, C), np.float64)
        for j in range(B):
            acc += summ[:, w * cfg.capp + j, :]
        for v in range(cfg.n_ov):
            for p in range(128):
                rr = idsrel[p, w, v]
                if 0 <= rr < W:
                    acc[int(rr)] += summ[p, w * cfg.capp + B + v]
        out[:, w, :] = acc * inv[:, w][:, None]
    return {"out": out.astype(ml_dtypes.bfloat16)}


def _install_axon_hooks_shim():
    """Provide antenv.axon_hooks + the ctypes NTFF hook if the image lacks it."""
    import contextlib
    import ctypes
    import types

    try:
        from antenv.axon_hooks import get_axon_ntff_profile_hook  # noqa: F401

        return
    except ImportError:
        pass
    import antenv

    mod = types.ModuleType("antenv.axon_hooks")
    state = {"h": None}
    mod.set_axon_ntff_profile_hook = lambda h: state.__setitem__("h", h)
    mod.get_axon_ntff_profile_hook = lambda: state["h"]
    antenv.axon_hooks = mod
    sys.modules["antenv.axon_hooks"] = mod

    so_path = "/opt/axon/libaxon_pjrt.so"
    if not os.path.exists(so_path):
        return
    lib = ctypes.CDLL(so_path)
    if not hasattr(lib, "axon_start_nrt_profile"):
        return
    lib.axon_start_nrt_profile.argtypes = [
        ctypes.POINTER(ctypes.c_int64),
        ctypes.c_size_t,
    ]
    lib.axon_start_nrt_profile.restype = ctypes.c_int64
    lib.axon_stop_nrt_profile.argtypes = [ctypes.c_char_p]
    lib.axon_stop_nrt_profile.restype = ctypes.c_int64

    @contextlib.contextmanager
    def _hook(output_dir, device_ids):
        import jax

        jax.devices()
        if device_ids:
            ids = (ctypes.c_int64 * len(device_ids))(*device_ids)
            rc = lib.axon_start_nrt_profile(ids, len(device_ids))
        else:
            rc = lib.axon_start_nrt_profile(None, 0)
        if rc != 0:
            raise RuntimeError(f"axon_start_nrt_profile rc={rc}")
        try:
            yield
        finally:
            n = lib.axon_stop_nrt_profile(str(output_dir).encode())
            print(f"profile: {n} file(s) written to {output_dir}", file=sys.stderr)

    state["h"] = _hook


def kernel(fine_feats, coarse_ids, num_coarse):
    global LAST_RESULT
    from concourse.bass_utils import run_bass_kernel_spmd

    num_coarse = int(num_coarse)
    pad = NCORES * W
    n_pad = max(250_880, -(-num_coarse // pad) * pad)
    cfg = Cfg(n_coarse_pad=n_pad)
    while cfg.n_win % cfg.chunk_windows != 0:
        n_pad += pad
        cfg = Cfg(n_coarse_pad=n_pad)

    in_maps, need_ov = shard_inputs(fine_feats, coarse_ids, cfg)
    while need_ov != cfg.n_ov:
        cfg = Cfg(n_coarse_pad=cfg.n_coarse_pad, n_ov=need_ov)
        in_maps, need_ov = shard_inputs(fine_feats, coarse_ids, cfg)

    key = ("v3", cfg.n_coarse_pad, cfg.n_ov)
    if key not in _nc_cache:
        _nc_cache[key] = build_nc(cfg)
    nc = _nc_cache[key]
    trace = bool(int(os.environ.get("KERNEL_TRACE", "0")))
    if trace:
        _install_axon_hooks_shim()
    res = run_bass_kernel_spmd(nc, in_maps, core_ids=list(range(NCORES)), trace=trace)
    LAST_RESULT = res
    return assemble_output(res.results, num_coarse, cfg)
